# revision 18
# baseline (speedup 1.0000x reference)
"""Causal GQA attention (paged-KV prefill) distributed over 8 TRN2 NeuronCores.

Problem: q [4096,16,128], k/v [4096,4,128] packed as B=2 sequences of S=2048,
KV-cache scatter via slot_mapping then causal attention (GQA group 4).

Sharding: 8 cores = (B=2) x (Hkv=4). Core c handles batch c//4, kv-head c%4
with its 4 query heads. No cross-core communication needed.

v2: full (below-diagonal) score groups run on the PE in fp8-e4m3 DoubleRow
mode at 0.5 cycles/col: stationary = host-packed (k8, dk8) residual pair
(k to ~0.1%), moving = host-duplicated (q8, q8), so scores = (k8+dk8)@q8 --
only the q side is one-sided-quantized (~3.6%/elem), and since every key in
a row shares the same dq, the row-mean component of the error cancels in
softmax normalization (measured ~1.3e-2 end-to-end with everything else).
Diagonal groups keep the bf16 path (short rows are error-sensitive).

Per-core kernel (Bass/Tile), inherited structure from the bf16 baseline:
  - host pre-casts/pre-transposes all operands; staged full-line-rate DMA
    prefixes in first-use order
  - scores^T tile [k=128, q<=512] on TensorE; causally trimmed diag tiles
  - exp on ScalarE from PSUM; every 3rd full group exps on VectorE as an
    int16 Schraudolph affine (bits = A*(scale*s)+B ~ bf16(exp))
  - diag masks via 0/1 tri mult on GpSimd (Pool)
  - out accumulation: psum_o[q=128,129] += probT.T @ [v|1]; 129th col
    accumulates the denominator; two q-subblocks pack per PSUM bank
  - normalize on DVE (reciprocal + tensor_scalar), staged PSUM->SBUF copy
  - software-pipelined emission with a 4-unit scores lookahead; PE clock
    warm-up during the input DMA fill
"""

import os
import sys

import numpy as np

for _p in ("/opt/trn_rl_repo",):
    if os.path.isdir(_p) and _p not in sys.path:
        sys.path.insert(0, _p)

import ml_dtypes  # noqa: E402

from concourse import bass, bacc, mybir, tile  # noqa: E402
from concourse.bass_utils import run_bass_kernel_spmd  # noqa: E402

B, S, H, HKV, D = 2, 2048, 16, 4, 128
GRP = H // HKV  # query heads per kv head
NCORES = 8
ST = S // 128  # 16 k-tiles of 128
QB = S // 512  # 4 q-chunks of 512
SCALE = 0.08838834764831845  # 1/sqrt(128)
import math as _math

A16S = (2.0**7) / _math.log(2.0) * SCALE
B16 = 127.0 * 2**7 - 366393.0 / 2**16

# --- schedule tuning knobs ---
PS_BUFS = 3
PO_BUFS = 1
# per-group exp-engine/dtype roles:
#   a8 = ACT exp -> e4m3 probs (x2^-4), fp8 DoubleRow AV with v*16 pairs
#   act = ACT exp -> bf16 probs, bf16 AV
#   dve/pool = Schraudolph int16-affine bf16 probs, bf16 AV
# NOTE: Pool (GPSIMD) cannot read PSUM on real HW, so exp roles are ACT/DVE
# only; Pool handles the SBUF-side masks and normalize multiplies.
FULL_PAT = ("a8", "a8", "dve", "a8", "a8", "dve", "a8", "a8", "dve", "a8", "a8", "dve")
DIAG_ROLE = {0: ("act", "act"), 1: ("dve", "dve"), 2: ("dve", "dve"), 3: ("dve", "dve")}
NORM_POOL = True  # normalize multiplies on Pool (SBUF-only op)
NORM_ACT_EVERY = 0  # every Nth chunk normalizes ACT-direct from PSUM
FULL_PAT_H0 = None  # optional distinct full-group pattern for head 0
MASK_PE = True  # fold diag masks into the score PSUM via a rank-127
# matmul of host consts: s += -B * 1[k > q]; exp then yields ~0 above the
# diagonal (e^-30), so no probT mask op is needed and Pool is freed.
MASK_B = 30.0 / SCALE  # raw-score offset; e^-30 ~ 1e-13, harmless in sums
MASK_POOL = True
MASK_DVE_QBS = ()
MASK_ALT = 0
BIAS8 = -4.0 * _math.log(2.0)  # fp8 prob pre-scale 2^-4 (v pairs carry x16)
NORM_MODE = "copy"
TAIL_FAST = True
WARMUP_N = 80
WARMUP_W = 32
WARMUP_PO = True
WARMUP_CONST = 100
WMM_DVE = True
TAIL_NORM_ACT = 0
TAIL_SPLIT_J = False
DRAIN_SMALL = True
LOOKAHEAD = 4
LAST_ORDER = (3, 1, 0, 2)
TAIL_KEEP = 1
TAIL_ACT_SPLIT = False
BANKCOPY_TAIL = 4

F32 = mybir.dt.float32
BF16 = mybir.dt.bfloat16
I16 = mybir.dt.int16
FP8E4 = mybir.dt.float8e4
DR = mybir.MatmulPerfMode.DoubleRow

_CACHED_NC = None

# ---------------------------------------------------------------------------
# bf16 kq layout (diag groups only): 16 kT tiles in diag-first-use order
# (t12..15, t8..11, t4..7, t0..3), then 16 qT 512-col chunks in (h, qb desc)
# order. Unit = 128 columns.
_K_SLOT2 = {t: (15 - t) // 4 * 4 + (t % 4) for t in range(ST)}
# t12->0..3? compute: t=12..15 -> (3)//4=0*4 + t%4 = 0..3; t8..11 -> 4..7;
# t4..7 -> 8..11; t0..3 -> 12..15.


def _kcol(t):
    return _K_SLOT2[t] * 128


def _qslot(h, qb):
    return h * 4 + (3 - qb)


def _qbase(h, qb):
    """bf16 qT column for head h chunk qb (after the 16 kT tiles)."""
    return (16 + _qslot(h, qb) * 4) * 128


KQ_COLS = 80 * 128  # 16 kT + 16 qT chunks of 4 tiles

# fp8 arrays: k8p tiles 0..11 (tiles 12-15 are never full); q8d slots for
# (h, qb in 3,2,1) since qb0 has no full groups.
NK8 = 12


def _q8slot(h, qb):
    return h * 3 + (3 - qb)


NQ8 = 12
NV8 = 12  # fp8 v pairs for tiles 0..11 (fp8 AV is full-tile only)


def _roles():
    """Static exp-engine role per (h, qb, g), mirroring the emission order."""
    roles = {}
    fc = 0  # full-group counter
    for h in range(GRP):
        for qb in (3, 2, 1, 0) if h < GRP - 1 else LAST_ORDER:
            n = 2 * qb + 2
            for g in range(n):
                if 2 * g + 1 < 4 * qb:  # full group
                    pat = FULL_PAT_H0 if (h == 0 and FULL_PAT_H0) else FULL_PAT
                    roles[(h, qb, g)] = pat[fc % len(pat)]
                    fc += 1
                else:
                    roles[(h, qb, g)] = DIAG_ROLE[qb][g - 2 * qb]
    return roles


ROLES = _roles()

# staged input DMA plan, first-use order. kinds:
#   k8: k8p tile range [a,b)   q8: q8d slot a   kq: bf16 col-tile range
#   v: v tile range            tri: mask
def _dma_plan():
    """First-use-ordered staged input plan derived from ROLES/order.
    Pieces: f0 (k8 t0,t1 + q8 h0qb3), k8 pair-of-tiles, q8 slot, v8 pairs,
    v bf16 pairs, kq 4-tile blocks (kT quads / qT chunks), mk consts."""
    order_ = [
        (h, qb)
        for h in range(GRP)
        for qb in ((3, 2, 1, 0) if h < GRP - 1 else LAST_ORDER)
    ]
    seq = []  # (need_index, kind, a, b)
    i = 0
    for h, qb in order_:
        for g in range(2 * qb + 2):
            kbs = (2 * g, 2 * g + 1)
            full = kbs[1] < 4 * qb
            role = ROLES[(h, qb, g)]
            if full:
                for kb in kbs:
                    if kb >= 2:
                        seq.append((i, "k8", kb, kb + 1))
                if not (h == 0 and qb == 3):
                    seq.append((i, "q8", _q8slot(h, qb), 0))
            else:
                for kb in kbs:
                    seq.append((i, "kq", _kcol(kb) // 128, _kcol(kb) // 128 + 1))
                seq.append((i, "kq", _qbase(h, qb) // 128, _qbase(h, qb) // 128 + 4))
                seq.append((i, "mk", 0, 0))
            for kb in kbs:
                if role == "a8":
                    seq.append((i + LOOKAHEAD, "v8", kb, kb + 1))
                else:
                    seq.append((i + LOOKAHEAD, "v", kb, kb + 1))
            i += 1
    first = {}
    for idx, kind, a, b in seq:
        for x in range(a, b if kind not in ("q8", "mk") else a + 1):
            key = (kind, x)
            if key not in first:
                first[key] = idx
    items = sorted(first.items(), key=lambda kv: kv[1])
    # merge same-kind range-contiguous pieces even when other kinds
    # interleave in time (cap 8 units); order pieces by first element's use
    open_p = {}  # kind -> [start, end, first_idx]
    plan = []
    for (kind, x), idx in items:
        if kind == "mk":
            plan.append((idx, ("mk", 0, 0)))
            continue
        if kind == "q8":
            plan.append((idx, ("q8", x, 0)))
            continue
        op = open_p.get(kind)
        if op and x == op[1] and op[1] - op[0] < 8:
            op[1] = x + 1
        else:
            if op:
                plan.append((op[2], (kind, op[0], op[1])))
            open_p[kind] = [x, x + 1, idx]
    for kind, op in open_p.items():
        plan.append((op[2], (kind, op[0], op[1])))
    plan.sort(key=lambda t: t[0])
    return [("f0", 0, 0)] + [p_ for _, p_ in plan]



def _chunk_plan(qb):
    """Group order for a chunk plus per-bank AV start/stop flags and the
    group after which each out bank completes."""
    n = 2 * qb + 2
    gorder = list(range(n))
    avs = []
    for g in gorder:
        for kb in (2 * g, 2 * g + 1):
            j0 = max(0, kb - 4 * qb)
            diag = kb >= 4 * qb
            js = list(range(j0 + 1, 4)) + [j0] if diag else list(range(4))
            for j in js:
                avs.append((g, kb, j))
    first, last = {}, {}
    for trip in avs:
        b = trip[2] // 2
        first.setdefault(b, trip)
        last[b] = trip
    starts = {first[0], first[1]}
    stops = {last[0], last[1]}
    normg = {0: last[0][0], 1: last[1][0]}
    return gorder, starts, stops, normg


def _build_graph():
    nc = bacc.Bacc(
        "TRN2", target_bir_lowering=False, debug=False, num_devices=NCORES
    )
    kq_ext = nc.declare_dram_parameter("kq", [128, KQ_COLS], BF16, isOutput=False)
    f0_ext = nc.declare_dram_parameter("f0", [128, 1024], FP8E4, isOutput=False)
    mk_ext = nc.declare_dram_parameter("mk", [128, 2, 128], BF16, isOutput=False)
    k8_ext = nc.declare_dram_parameter("k8p", [128, NK8, 2, 128], FP8E4, isOutput=False)
    q8_ext = nc.declare_dram_parameter("q8d", [128, NQ8, 512], FP8E4, isOutput=False)
    v8_ext = nc.declare_dram_parameter("v8p", [128, NV8, 2, 129], FP8E4, isOutput=False)
    v_ext = nc.declare_dram_parameter("v", [128, ST, D + 1], BF16, isOutput=False)
    tri_ext = nc.declare_dram_parameter("tri", [128, 128], BF16, isOutput=False)
    out_ext = nc.declare_dram_parameter("out", [S, GRP, D], BF16, isOutput=True)

    with tile.TileContext(nc) as tc:
        with (
            tc.tile_pool(name="const", bufs=1) as constp,
            tc.tile_pool(name="kv", bufs=1) as kvp,
            tc.tile_pool(name="prob", bufs=10) as probp,
            tc.tile_pool(name="osb", bufs=8) as osbp,
            tc.tile_pool(name="small", bufs=16) as smallp,
            tc.tile_pool(name="ps_s", bufs=PS_BUFS, space=bass.MemorySpace.PSUM) as pss,
            tc.tile_pool(name="ps_o", bufs=PO_BUFS, space=bass.MemorySpace.PSUM) as pso,
        ):
            tri = constp.tile([128, 128], BF16)
            mk = constp.tile([128, 2, 128], BF16, tag="mk")
            um = mk[:, 0]
            wm = mk[:, 1]
            f0 = constp.tile([128, 1024], FP8E4, tag="f0")
            b8c = constp.tile([128, 1], F32, tag="b8c")
            kq = kvp.tile([128, KQ_COLS], BF16, tag="kq")
            kqf = kq[:]
            k8p = kvp.tile([128, NK8, 2, 128], FP8E4, tag="k8p")
            q8d = kvp.tile([128, NQ8, 512], FP8E4, tag="q8d")
            v8p = kvp.tile([128, NV8, 2, 129], FP8E4, tag="v8p")
            v_aug = kvp.tile([128, ST, 129], BF16, tag="vaug")
            v_augf = v_aug[:].rearrange("s0 st d -> s0 (st d)")

            for kind, a, b_ in _dma_plan():
                if kind == "kq":
                    nc.sync.dma_start(
                        kq[:, a * 128 : b_ * 128], kq_ext.ap()[:, a * 128 : b_ * 128]
                    )
                elif kind == "k8":
                    nc.sync.dma_start(k8p[:, a:b_], k8_ext.ap()[:, a:b_])
                elif kind == "q8":
                    nc.sync.dma_start(
                        q8d[:, a : a + 1], q8_ext.ap()[:, a : a + 1]
                    )
                elif kind == "v8":
                    nc.sync.dma_start(v8p[:, a:b_], v8_ext.ap()[:, a:b_])
                elif kind == "v":
                    nc.sync.dma_start(v_aug[:, a:b_, :], v_ext.ap()[:, a:b_, :])
                elif kind == "mk":
                    nc.sync.dma_start(mk[:], mk_ext.ap())
                elif kind == "f0":
                    nc.sync.dma_start(f0[:], f0_ext.ap())
                else:
                    nc.sync.dma_start(tri[:], tri_ext.ap())

            # warm the exp table set while input DMAs run
            warm = smallp.tile([128, 1], F32, tag="warm")
            nc.vector.memset(b8c[:], BIAS8)
            nc.vector.memset(warm[:], 0.0)
            nc.scalar.activation(
                warm[:], warm[:], mybir.ActivationFunctionType.Exp
            )
            # warm the PE clock during the input fill
            wmm = smallp.tile([128, WARMUP_W], BF16, tag="wmm")
            if WARMUP_CONST:
                cap = nc.const_aps.tensor(1.0, (128, 1), BF16)
                cps = (
                    pso.tile([128, 258], F32, tag="o01", name="wpsc")
                    if WARMUP_PO
                    else pss.tile([128, 1024], F32, tag="s", name="wpsc")
                )
                for _ in range(WARMUP_CONST):
                    nc.tensor.matmul(
                        cps[:1, 0:1], cap, cap, start=True, stop=True
                    )
            (nc.vector if WMM_DVE else nc.gpsimd).memset(wmm[:], 0.0)
            if WARMUP_PO:
                wps = pso.tile([128, 258], F32, tag="o01", name="wps")
            else:
                wps = pss.tile([128, 1024], F32, tag="s", name="wps")
            for _ in range(WARMUP_N):
                nc.tensor.matmul(
                    wps[:WARMUP_W, 0:WARMUP_W],
                    wmm[:],
                    wmm[:],
                    start=True,
                    stop=True,
                )

            outr = out_ext.ap().rearrange(
                "(qb bk jj s0) h d -> qb h bk s0 jj d", bk=2, jj=2, s0=128
            )
            outr4 = out_ext.ap().rearrange(
                "(qb j s0) h d -> qb h s0 j d", j=4, s0=128
            )

            def po_slice(po, j):
                t = po[0] if j < 2 else po[1]
                off = 129 * (j % 2)
                return t[:, off : off + 129]

            def emit_scores(h, qb, g):
                """Issue the score matmuls for k-tile pair g. Full groups:
                fp8 DoubleRow (k8,dk8)@(q8,q8) at 0.5 cyc/col."""
                kbs = (2 * g, 2 * g + 1)
                full = kbs[1] < 4 * qb
                ps = pss.tile([128, 1024], F32, tag="s", name="ps")
                if full:
                    qs = _q8slot(h, qb)
                    first0 = h == 0 and qb == 3
                    qsrc = f0[:, 512:1024] if first0 else q8d[:, qs]
                    qmv = qsrc.unsqueeze(1).broadcast_to([128, 2, 512])
                    for i in (0, 1):
                        # k-pair tiles 0,1 live in the merged first blob for
                        # ALL chunks (k8p[:, 0:2] is never DMA'd separately)
                        kst = (
                            f0[:, kbs[i] * 256 : kbs[i] * 256 + 256].rearrange(
                                "p (two k) -> p two k", two=2
                            )
                            if kbs[i] < 2
                            else k8p[:, kbs[i]]
                        )
                        nc.tensor.matmul(
                            ps[:, i * 512 : (i + 1) * 512],
                            kst,
                            qmv,
                            start=True,
                            stop=True,
                            perf_mode=DR,
                        )
                    return (ps, kbs, [0, 0], [0, 512], [512, 512])
                trims = [max(0, kb - 4 * qb) * 128 for kb in kbs]
                widths = [512 - t for t in trims]
                same_bank = widths[0] + widths[1] <= 512
                offs = [0, widths[0]] if same_bank else [0, 512]
                qstart = _qbase(h, qb)
                for i in (0, 1):
                    kb, t, w, o = kbs[i], trims[i], widths[i], offs[i]
                    nc.tensor.matmul(
                        ps[:, o : o + w],
                        kqf[:, _kcol(kb) : _kcol(kb) + 128],
                        kqf[:, qstart + t : qstart + 512],
                        start=(not same_bank) or i == 0,
                        stop=(not same_bank) and not MASK_PE,
                        skip_group_check=MASK_PE,
                    )
                if MASK_PE:
                    # accumulate -B above the diagonal of each tile's first
                    # (partial) 128-col block: out[k,q] += sum_m um[m,k]*wm[m,q]
                    for i in (0, 1):
                        nc.tensor.matmul(
                            ps[:, offs[i] : offs[i] + 128],
                            um[:],
                            wm[:],
                            start=False,
                            stop=i == 1,
                            skip_group_check=True,
                        )
                return (ps, kbs, trims, offs, widths)

            def norm_and_store(
                po, bk, h, qb, staged=True, act_split=False, act_all=False
            ):
                if staged:
                    acc = osbp.tile([128, 258], F32, tag="acc", name="acc")
                    nc.vector.tensor_copy(acc[:], po[bk][:])
                    src = acc
                else:
                    src = po[bk]
                out_sb = osbp.tile([128, 2, 128], BF16, tag="out", name="osb")
                rcps = []
                for jj in (0, 1):
                    aj = src[:, 129 * jj : 129 * jj + 129]
                    rcp = smallp.tile([128, 1], F32, tag="rcp", name="rcp")
                    nc.vector.reciprocal(rcp[:], aj[:, 128:129])
                    rcps.append((aj, rcp))
                for jj, (aj, rcp) in enumerate(rcps):
                    if act_all or (jj == 0 and act_split):
                        nc.scalar.activation(
                            out_sb[:, jj, :],
                            aj[:, 0:128],
                            mybir.ActivationFunctionType.Copy,
                            scale=rcp[:],
                        )
                    elif NORM_POOL and staged:
                        nc.gpsimd.tensor_scalar_mul(
                            out_sb[:, jj, :], aj[:, 0:128], rcp[:]
                        )
                    else:
                        nc.vector.tensor_scalar_mul(
                            out_sb[:, jj, :], aj[:, 0:128], rcp[:]
                        )
                nc.sync.dma_start(outr[qb, h, bk], out_sb[:])

            def emit_rest(h, qb, g, po, plan, scored):
                ps, kbs, trims, offs, widths = scored
                role = ROLES[(h, qb, g)]
                total_w = offs[1] + widths[1]
                fp8av = role == "a8"
                if role in ("dve", "pool"):
                    i16 = probp.tile([128, 1024], I16, tag="p", name="probTi")
                    nc.vector.tensor_scalar(
                        i16[:, 0:total_w],
                        ps[:, 0:total_w],
                        A16S,
                        B16,
                        mybir.AluOpType.mult,
                        mybir.AluOpType.add,
                    )
                    probT = i16.bitcast(BF16)
                elif fp8av:
                    probT8_t = probp.tile(
                        [128, 1024], FP8E4, tag="p8", name="probT8"
                    )
                    probT = probT8_t[:]
                    nc.scalar.activation(
                        probT[:, 0:total_w],
                        ps[:, 0:total_w],
                        mybir.ActivationFunctionType.Exp,
                        scale=SCALE,
                        bias=b8c[:],
                    )
                else:
                    probT_t = probp.tile(
                        [128, 1024], BF16, tag="p", name="probT"
                    )
                    probT = probT_t[:]
                    nc.scalar.activation(
                        probT[:, 0:total_w],
                        ps[:, 0:total_w],
                        mybir.ActivationFunctionType.Exp,
                        scale=SCALE,
                    )
                exp_state["unit"] += 1
                _, starts, stops, normg = plan
                for i in (0, 1):
                    kb, t, o = kbs[i], trims[i], offs[i]
                    j0 = t // 128
                    diag = kb >= 4 * qb
                    if diag and not MASK_PE:
                        blk = probT[:, o : o + 128]
                        exp_state["mask"] += 1
                        on_pool = MASK_POOL and qb not in MASK_DVE_QBS
                        if on_pool and MASK_ALT and (
                            exp_state["mask"] % MASK_ALT == 0
                        ):
                            on_pool = False
                        if on_pool:
                            nc.gpsimd.tensor_mul(blk, blk, tri[:])
                        else:
                            nc.vector.tensor_mul(blk, blk, tri[:])
                    js = list(range(j0 + 1, 4)) + [j0] if diag else range(4)
                    for j in js:
                        co = o + (j - j0) * 128
                        if fp8av:
                            # fp8 DoubleRow AV: probs broadcast over the
                            # (v8, dv) residual pair; ones-col carries x16
                            nc.tensor.matmul(
                                po_slice(po, j),
                                probT[:, co : co + 128]
                                .unsqueeze(1)
                                .broadcast_to([128, 2, 128]),
                                v8p[:, kb],
                                start=(g, kb, j) in starts,
                                stop=(g, kb, j) in stops,
                                perf_mode=DR,
                                skip_group_check=True,
                            )
                        else:
                            nc.tensor.matmul(
                                po_slice(po, j),
                                probT[:, co : co + 128],
                                v_augf[:, kb * 129 : (kb + 1) * 129],
                                start=(g, kb, j) in starts,
                                stop=(g, kb, j) in stops,
                                skip_group_check=True,
                            )
                ui = order.index((h, qb))
                last_chunk = ui == len(order) - 1
                tail_bankcopy = TAIL_FAST and ui >= len(order) - BANKCOPY_TAIL
                act_direct = NORM_ACT_EVERY and ui % NORM_ACT_EVERY == 0
                if act_direct and not tail_bankcopy and not last_chunk:
                    for bk in (0, 1):
                        if g == normg[bk]:
                            norm_and_store(
                                po, bk, h, qb, staged=False, act_all=True
                            )
                elif NORM_MODE in ("direct", "bankcopy") or tail_bankcopy:
                    staged = NORM_MODE == "bankcopy" or tail_bankcopy
                    if g == normg[0]:
                        norm_and_store(po, 0, h, qb, staged=staged)
                    if g == normg[1]:
                        fast = TAIL_FAST and last_chunk
                        norm_and_store(
                            po,
                            1,
                            h,
                            qb,
                            staged=staged and not fast,
                            act_split=fast and TAIL_ACT_SPLIT,
                            act_all=ui >= len(order) - TAIL_NORM_ACT,
                        )
                elif g == normg[1]:
                    acc = osbp.tile([128, 2, 258], F32, tag="acc", name="acc")
                    nc.vector.tensor_copy(acc[:, 0, :], po[0][:])
                    nc.vector.tensor_copy(acc[:, 1, :], po[1][:])
                    out_sb = osbp.tile(
                        [128, 4, 128], BF16, tag="out", name="osb"
                    )
                    for j in range(4):
                        aj = acc[:, j // 2, 129 * (j % 2) : 129 * (j % 2) + 129]
                        rcp = smallp.tile([128, 1], F32, tag="rcp", name="rcp")
                        nc.vector.reciprocal(rcp[:], aj[:, 128:129])
                        if NORM_POOL:
                            nc.gpsimd.tensor_scalar_mul(
                                out_sb[:, j, :], aj[:, 0:128], rcp[:]
                            )
                        else:
                            nc.vector.tensor_scalar_mul(
                                out_sb[:, j, :], aj[:, 0:128], rcp[:]
                            )
                    nc.sync.dma_start(outr4[qb, h], out_sb[:])

            exp_state = {"ctr": 0, "unit": 0, "mask": 0}
            pending = []
            order = [
                (h, qb)
                for h in range(GRP)
                for qb in ((3, 2, 1, 0) if h < GRP - 1 else LAST_ORDER)
            ]
            for h, qb in order:
                    if qb == 0 and DRAIN_SMALL:
                        keep = TAIL_KEEP if h == GRP - 1 else 2
                        while len(pending) > keep:
                            emit_rest(*pending.pop(0))
                    po01 = pso.tile([128, 258], F32, tag="o01", name="po01")
                    po23 = pso.tile([128, 258], F32, tag="o23", name="po23")
                    po = (po01, po23)
                    plan = _chunk_plan(qb)
                    for g in plan[0]:
                        scored = emit_scores(h, qb, g)
                        pending.append((h, qb, g, po, plan, scored))
                        if len(pending) > LOOKAHEAD:
                            emit_rest(*pending.pop(0))
            for p in pending:
                emit_rest(*p)

    nc.compile()
    return nc


def _get_nc():
    global _CACHED_NC
    if _CACHED_NC is None:
        _CACHED_NC = _build_graph()
    return _CACHED_NC


def _effective_kv(kv, cache, slot):
    valid = slot >= 0
    safe = np.where(valid, slot, 0)
    cache = np.array(cache, dtype=np.float32, copy=True)
    val = np.where(valid[:, None, None], kv, cache[safe])
    cache[safe] = val
    return cache[safe.reshape(B, S)]


def _tile_sd(x):
    S_, D_ = x.shape
    return np.ascontiguousarray(
        x.reshape(S_ // 128, 128, D_).transpose(1, 0, 2)
    )


def _prep_core_inputs(qb, kk, vv, tri, c):
    bf16 = ml_dtypes.bfloat16
    e4 = ml_dtypes.float8_e4m3fn
    b, g = c // HKV, c % HKV
    q_sh = qb[b, :, g * GRP : (g + 1) * GRP, :].astype(bf16)  # [S, GRP, D]
    k_sh = kk[b, :, g, :].astype(bf16)  # [S, D]
    kq = np.empty((128, KQ_COLS), dtype=bf16)
    kT = np.ascontiguousarray(k_sh.T)  # [128 d, S]
    for t in range(ST):
        kq[:, _kcol(t) : _kcol(t) + 128] = kT[:, t * 128 : (t + 1) * 128]
    qTs = {}
    for h in range(GRP):
        qT = np.ascontiguousarray(q_sh[:, h, :].T)  # [128 d, S]
        qTs[h] = qT
        for qbi in range(QB):
            c0 = _qbase(h, qbi)
            kq[:, c0 : c0 + 512] = qT[:, qbi * 512 : (qbi + 1) * 512]
    # fp8 K pairs (k8, dk8), tiles 0..11
    kTf = kT.astype(np.float32)
    k8 = kTf.astype(e4)
    dk8 = (kTf - k8.astype(np.float32)).astype(e4)
    k8p = np.empty((128, NK8, 2, 128), dtype=e4)
    for t in range(NK8):
        k8p[:, t, 0, :] = k8[:, t * 128 : (t + 1) * 128]
        k8p[:, t, 1, :] = dk8[:, t * 128 : (t + 1) * 128]
    # fp8 q8 single copy (broadcast to both DoubleRow slices on-chip)
    q8d = np.empty((128, NQ8, 512), dtype=e4)
    for h in range(GRP):
        q8 = qTs[h].astype(np.float32).astype(e4)
        for qbi in (3, 2, 1):
            q8d[:, _q8slot(h, qbi), :] = q8[:, qbi * 512 : (qbi + 1) * 512]
    v_sd = vv[b, :, g, :].astype(bf16)  # [S, D]
    v_pad = np.concatenate(
        [v_sd, np.ones((S, 1), dtype=bf16)], axis=1
    )
    v_tiled = _tile_sd(v_pad)  # [128, ST, 129]
    # fp8 v pairs: (e4m3(16*v) | 16), (e4m3(16v - v8) | 0) for tiles 0..11
    vf = v_tiled.astype(np.float32) * 16.0  # [128, ST, 129] incl ones col
    v8 = vf.astype(e4)
    dv8 = (vf - v8.astype(np.float32)).astype(e4)
    v8p = np.empty((128, NV8, 2, 129), dtype=e4)
    v8p[:, :, 0, :] = v8[:, :NV8, :]
    v8p[:, :, 1, :] = dv8[:, :NV8, :]
    m = np.arange(128)
    um = (m[None, :] > m[:, None]).astype(np.float32).astype(bf16)  # U[m,k]=1[k>m]
    wm = (-MASK_B * np.eye(128, dtype=np.float32)).astype(bf16)
    mk = np.stack([um, wm], axis=1)  # [128, 2, 128]
    f0 = np.empty((128, 1024), dtype=e4)
    f0[:, 0:256] = k8p[:, 0].reshape(128, 256)
    f0[:, 256:512] = k8p[:, 1].reshape(128, 256)
    f0[:, 512:1024] = q8d[:, _q8slot(0, 3)]
    return {"kq": kq, "k8p": k8p, "q8d": q8d, "v8p": v8p, "v": v_tiled,
            "tri": tri, "mk": mk, "f0": f0}


def kernel(q, k, v, k_cache, v_cache, slot_mapping, batch, seqlen, **_ignored):
    q = np.asarray(q, dtype=np.float32)
    k = np.asarray(k, dtype=np.float32)
    v = np.asarray(v, dtype=np.float32)
    slot = np.asarray(slot_mapping).astype(np.int64)
    assert int(batch) == B and int(seqlen) == S
    assert q.shape == (B * S, H, D)

    kk = _effective_kv(k, k_cache, slot)  # [B, S, HKV, D]
    vv = _effective_kv(v, v_cache, slot)
    qb = q.reshape(B, S, H, D)

    tri = np.triu(np.ones((128, 128), dtype=np.float32)).astype(
        ml_dtypes.bfloat16
    )

    in_maps = [
        _prep_core_inputs(qb, kk, vv, tri, c) for c in range(NCORES)
    ]

    nc = _get_nc()
    res = run_bass_kernel_spmd(nc, in_maps, core_ids=list(range(NCORES)))

    out = np.empty((B, S, H, D), dtype=np.float32)
    for c in range(NCORES):
        b, g = c // HKV, c % HKV
        out[b, :, g * GRP : (g + 1) * GRP, :] = res.results[c]["out"].astype(
            np.float32
        )
    return out.reshape(B * S, H, D)
